# revision 8
# baseline (speedup 1.0000x reference)
"""Trainium2 Bass kernel for causal MHA (B=32, T=576, C=1024, H=16).

Data-parallel over batch across 8 NeuronCores (4 batches/core), all
matmuls in bf16 on the tensor engine (fp32 PSUM accumulation).

The wall-clock cost of this problem is dominated by the axon tunnel
(~40-100 MB/s host<->device), so the host side is organized around
minimizing wire bytes per call:
  - x and out cross the wire as bf16 in natural [tokens, C] layout
    (transposes to/from the feature-major compute layout happen on the
    tensor engine via identity matmuls, which is ~free on-device).
  - weights/biases are uploaded once (bf16) and kept device-resident
    across calls; a host-side equality check invalidates the cache if
    the caller passes different weights.
  - the donated zero output buffers are created on-device by a tiny
    jitted zeros function instead of being shipped from the host.
  - the NEFF-wrapping jitted function is built/compiled once and
    reused (the stock run path re-traces and re-lowers per call).
  - h2d/d2h use per-shard transfers on an 8-thread pool (the tunnel
    gives ~2-3x aggregate over a single stream).

Per-core dataflow (per batch, 576 tokens):
  - x [576, C] bf16 -> xT [C, 576] via identity matmuls.
  - q,k feature-major: qkT[n, t] = w_qkv[:, n].T @ xT (w stationary).
  - v token-major with a ones-column per head (v' = [v_h | 1]) so the
    same matmul accumulates softmax denominators.
  - scores.T[j, i] = k_h[d, j].T @ q_h[d, i]; exp via ScalarE
    (scale 1/64); causal mask via gpsimd affine_select (zero j > i).
  - y.T[d, i] (+ denom row) = v'_h.T @ att.T, PSUM-accumulated;
    normalize with DVE reciprocal + partition_broadcast + mul.
  - outT[n, t] = w_proj[:, n].T @ yT + bias; transpose back to
    [576, C] via identity matmuls; DMA out as bf16.
"""

import numpy as np
import ml_dtypes

import concourse.bass as bass
import concourse.mybir as mybir
import concourse.tile as tile
from concourse import bacc

B, T, C, H = 32, 576, 1024, 16
D = C // H            # 64
NCORES = 8
BPC = B // NCORES     # 4 batches per core
M = BPC * T           # 2304 tokens per core

F32 = mybir.dt.float32
BF16 = mybir.dt.bfloat16
AF = mybir.ActivationFunctionType
ALU = mybir.AluOpType
NPBF16 = ml_dtypes.bfloat16

KC = C // 128         # 8 contraction chunks
NT_QK = 16            # q/k feature tiles of 128 (q: 0-7, k: 8-15)
NT_PROJ = 8
TT = [(t0, min(128, T - t0)) for t0 in range(0, T, 128)]   # token chunks
# score blocks: (j0, jw, i0, iw) — keys [j0, j0+jw), queries [i0, i0+iw)
SBLK = [
    (0,   128, 0,   576),
    (128, 128, 0,   576),
    (256, 128, 256, 320),
    (384, 128, 288, 288),
    (512, 64,  288, 288),
]
# x-transpose groups: (xt_col_offset, [(tile_idx, psum_offset, width), ...])
TGRP = [
    (0,   [(0, 0, 128), (1, 128, 128)]),
    (256, [(2, 0, 128), (3, 128, 128)]),
    (512, [(4, 0, 64)]),
]


def build_program():
    nc = bacc.Bacc(
        "TRN2", target_bir_lowering=False, debug=False,
        enable_asserts=False, num_devices=NCORES,
    )
    x_nat = nc.dram_tensor("x_nat", [M, C], BF16, kind="ExternalInput").ap()
    w_qkv = nc.dram_tensor("w_qkv", [C, 3 * C], BF16, kind="ExternalInput").ap()
    b_qkv = nc.dram_tensor("b_qkv", [3 * C], F32, kind="ExternalInput").ap()
    w_proj = nc.dram_tensor("w_proj", [C, C], BF16, kind="ExternalInput").ap()
    bvr = nc.dram_tensor("bvr", [1, C], BF16, kind="ExternalInput").ap()
    ones_r = nc.dram_tensor("ones_r", [1, 128], BF16, kind="ExternalInput").ap()
    ones_c = nc.dram_tensor("ones_c", [128, H], BF16, kind="ExternalInput").ap()
    ident = nc.dram_tensor("ident", [128, 128], BF16, kind="ExternalInput").ap()
    b_proj = nc.dram_tensor("b_proj", [C], F32, kind="ExternalInput").ap()
    out_nat = nc.dram_tensor("out_nat", [M, C], BF16, kind="ExternalOutput").ap()

    from contextlib import ExitStack
    with tile.TileContext(nc) as tc, ExitStack() as ctx:
        ep = ctx.enter_context
        # --- SBUF pools ---
        const_p = ep(tc.tile_pool(name="const", bufs=1))
        xn_p   = ep(tc.tile_pool(name="xn", bufs=len(TT) + 2))
        xt_p   = ep(tc.tile_pool(name="xt", bufs=KC + 2))
        qk_p   = ep(tc.tile_pool(name="qk", bufs=NT_QK + 2))
        vtm_p  = ep(tc.tile_pool(name="vtm", bufs=len(TT) + 1))
        att_p  = ep(tc.tile_pool(name="att", bufs=6))
        yt_p   = ep(tc.tile_pool(name="yt", bufs=KC + 1))
        osb_p  = ep(tc.tile_pool(name="osb", bufs=NT_PROJ + 1))
        onat_p = ep(tc.tile_pool(name="onat", bufs=3))
        rc_p   = ep(tc.tile_pool(name="rc", bufs=3))
        rb_p   = ep(tc.tile_pool(name="rb", bufs=3))
        # --- PSUM pools (8 banks total: 3 + 3 + 2) ---
        mm_ps  = ep(tc.tile_pool(name="mm_ps", bufs=3, space="PSUM"))
        s_ps   = ep(tc.tile_pool(name="s_ps", bufs=3, space="PSUM"))
        y_ps   = ep(tc.tile_pool(name="y_ps", bufs=2, space="PSUM"))

        # ---- constants: biases, ones, identity ----
        bqk_sb = const_p.tile([128, NT_QK], F32, tag="bqk", name="bqk")
        for nt in range(NT_QK):
            nc.sync.dma_start(
                bqk_sb[:, nt:nt + 1],
                b_qkv[nt * 128:(nt + 1) * 128].rearrange("(p o) -> p o", o=1),
            )
        bp_sb = const_p.tile([128, NT_PROJ], F32, tag="bp", name="bp")
        for nt in range(NT_PROJ):
            nc.sync.dma_start(
                bp_sb[:, nt:nt + 1],
                b_proj[nt * 128:(nt + 1) * 128].rearrange("(p o) -> p o", o=1),
            )
        bv_row = const_p.tile([1, C], BF16, tag="bv", name="bv")
        nc.sync.dma_start(bv_row[:, :], bvr[:, :])
        ones_row = const_p.tile([1, 128], BF16, tag="ones", name="ones")
        nc.sync.dma_start(ones_row[:, :], ones_r[:, :])
        id_sb = const_p.tile([128, 128], BF16, tag="id", name="id")
        nc.sync.dma_start(id_sb[:, :], ident[:, :])

        # ---- resident weights (loaded once, reused for all batches) ----
        wqk = []
        for kc in range(KC):
            t = const_p.tile([128, 2 * C], BF16, tag="wqk", name="wqk", bufs=KC)
            nc.sync.dma_start(t[:, :], w_qkv[kc * 128:(kc + 1) * 128, 0:2 * C])
            wqk.append(t)
        wv = []
        for kc in range(KC):
            t = const_p.tile([128, C], BF16, tag="wv", name="wv", bufs=KC)
            nc.sync.dma_start(t[:, :], w_qkv[kc * 128:(kc + 1) * 128, 2 * C:3 * C])
            wv.append(t)
        wp = []
        for kc in range(KC):
            t = const_p.tile([128, C], BF16, tag="wp", name="wp", bufs=KC)
            nc.sync.dma_start(t[:, :], w_proj[kc * 128:(kc + 1) * 128, :])
            wp.append(t)

        for b in range(BPC):
            mofs = b * T

            # ---- load x (natural layout) for this batch ----
            xn = []
            for (t0, tp) in TT:
                t = xn_p.tile([128, C], BF16, tag="xn", name="xn")
                nc.sync.dma_start(t[:tp, :], x_nat[mofs + t0:mofs + t0 + tp, :])
                xn.append(t)

            # ---- transpose x -> xT[cc] [128, T] via identity matmuls ----
            xt = []
            for cc in range(KC):
                t = xt_p.tile([128, T], BF16, tag="xt", name="xt")
                for gi, (coff, chunks) in enumerate(TGRP):
                    gw = sum(c[2] for c in chunks)
                    ps = mm_ps.tile([128, 288], F32, tag="mm", name="mm")
                    for (ti, off, tw) in chunks:
                        nc.tensor.matmul(
                            ps[:, off:off + tw],
                            xn[ti][0:tw, cc * 128:(cc + 1) * 128],
                            id_sb[0:tw, 0:tw],
                            start=True, stop=True)
                    dst = t[:, coff:coff + gw]
                    if gi % 2 == 0:
                        nc.scalar.activation(dst, ps[:, 0:gw], AF.Identity)
                    else:
                        nc.vector.tensor_copy(dst, ps[:, 0:gw])
                xt.append(t)

            # ---- QKV: q/k feature-major ----
            qk = []
            for nt in range(NT_QK):
                psA = mm_ps.tile([128, 288], F32, tag="mm", name="mm")
                psB = mm_ps.tile([128, 288], F32, tag="mm", name="mm")
                for kc in range(KC):
                    wt = wqk[kc][:, nt * 128:(nt + 1) * 128]
                    nc.tensor.matmul(psA[:, :], wt, xt[kc][:, 0:288],
                                     start=(kc == 0), stop=(kc == KC - 1))
                    nc.tensor.matmul(psB[:, :], wt, xt[kc][:, 288:576],
                                     start=(kc == 0), stop=(kc == KC - 1))
                qt = qk_p.tile([128, T], BF16, tag="qk", name="qk")
                bias = bqk_sb[:, nt:nt + 1]
                if nt < 8:   # q -> ScalarE copy w/ bias
                    nc.scalar.activation(qt[:, 0:288], psA[:, :], AF.Identity, bias=bias)
                    nc.scalar.activation(qt[:, 288:576], psB[:, :], AF.Identity, bias=bias)
                else:        # k -> VectorE copy w/ bias
                    nc.vector.tensor_scalar_add(qt[:, 0:288], psA[:, :], bias)
                    nc.vector.tensor_scalar_add(qt[:, 288:576], psB[:, :], bias)
                qk.append(qt)

            # ---- V token-major, with ones column per head (stride 65) ----
            vtm = []
            for (t0, tp) in TT:
                vt = vtm_p.tile([128, H * (D + 1)], BF16, tag="vtm", name="vtm")
                ones_cols = vt[:tp, :].rearrange("p (h e) -> p h e", e=D + 1)[:, :, D:D + 1]
                nc.sync.dma_start(ones_cols, ones_c[:tp, :].rearrange("p h -> p h ()"))
                vtm.append(vt)
            for nch in range(4):          # 256-wide chunks of the v columns
                for ti, (t0, tp) in enumerate(TT):
                    psV = mm_ps.tile([128, 288], F32, tag="mm", name="mm")
                    for kc in range(KC):
                        nc.tensor.matmul(psV[:tp, 0:256],
                                         xt[kc][:, t0:t0 + tp],
                                         wv[kc][:, nch * 256:(nch + 1) * 256],
                                         start=(kc == 0), stop=False)
                    nc.tensor.matmul(psV[:tp, 0:256],
                                     ones_row[:, :tp],
                                     bv_row[:, nch * 256:(nch + 1) * 256],
                                     start=False, stop=True)
                    for hh in range(4):
                        h = nch * 4 + hh
                        nc.vector.tensor_copy(
                            vtm[ti][:tp, h * 65:h * 65 + 64],
                            psV[:tp, hh * 64:(hh + 1) * 64],
                        )

            # ---- attention per head ----
            yt = [yt_p.tile([128, T], BF16, tag="yt", name="yt") for _ in range(KC)]
            for h in range(H):
                p0 = (h % 2) * 64
                qt = qk[h // 2]
                kt = qk[8 + h // 2]
                att = []
                for (j0, jw, i0, iw) in SBLK:
                    at = att_p.tile([jw, iw], BF16, tag="att", name="att")
                    for c0 in range(0, iw, 288):
                        cw = min(288, iw - c0)
                        sp = s_ps.tile([jw, cw], F32, tag="s", name="s")
                        nc.tensor.matmul(
                            sp[:, :],
                            kt[p0:p0 + 64, j0:j0 + jw],
                            qt[p0:p0 + 64, i0 + c0:i0 + c0 + cw],
                            start=True, stop=True)
                        nc.scalar.activation(at[:, c0:c0 + cw], sp[:, :],
                                             AF.Exp, scale=1.0 / D)
                    # zero where j > i:  keep iff (i0+f) - (j0+p) >= 0
                    mw = min(iw, j0 + jw - i0)   # cols that can be masked
                    if mw > 0:
                        nc.gpsimd.affine_select(
                            out=at[:, 0:mw], in_=at[:, 0:mw],
                            compare_op=ALU.is_ge, fill=0.0,
                            base=i0 - j0, channel_multiplier=-1,
                            pattern=[[1, mw]],
                        )
                    att.append(at)

                y0 = y_ps.tile([65, 288], F32, tag="y", name="y")
                y1 = y_ps.tile([65, 288], F32, tag="y", name="y")
                # columns i in [0, 288)
                nc.tensor.matmul(y0[:, :], vtm[0][:128, h * 65:h * 65 + 65],
                                 att[0][:, 0:288], start=True, stop=False)
                nc.tensor.matmul(y0[:, :], vtm[1][:128, h * 65:h * 65 + 65],
                                 att[1][:, 0:288], start=False, stop=False)
                nc.tensor.matmul(y0[:, 256:288], vtm[2][:128, h * 65:h * 65 + 65],
                                 att[2][:, 0:32], start=False, stop=True)
                # columns i in [288, 576)
                nc.tensor.matmul(y1[:, :], vtm[0][:128, h * 65:h * 65 + 65],
                                 att[0][:, 288:576], start=True, stop=False)
                nc.tensor.matmul(y1[:, :], vtm[1][:128, h * 65:h * 65 + 65],
                                 att[1][:, 288:576], start=False, stop=False)
                nc.tensor.matmul(y1[:, :], vtm[2][:128, h * 65:h * 65 + 65],
                                 att[2][:, 32:320], start=False, stop=False)
                nc.tensor.matmul(y1[:, :], vtm[3][:128, h * 65:h * 65 + 65],
                                 att[3][:, 0:288], start=False, stop=False)
                nc.tensor.matmul(y1[:, :], vtm[4][:64, h * 65:h * 65 + 65],
                                 att[4][:, 0:288], start=False, stop=True)

                rc = rc_p.tile([1, T], F32, tag="rc", name="rc")
                nc.vector.reciprocal(rc[:, 0:288], y0[64:65, :])
                nc.vector.reciprocal(rc[:, 288:576], y1[64:65, :])
                rb = rb_p.tile([64, T], F32, tag="rb", name="rb")
                nc.gpsimd.partition_broadcast(rb[:, :], rc[0:1, :])
                g = h // 2
                nc.vector.tensor_mul(yt[g][p0:p0 + 64, 0:288], y0[0:64, :], rb[:, 0:288])
                nc.vector.tensor_mul(yt[g][p0:p0 + 64, 288:576], y1[0:64, :], rb[:, 288:576])

            # ---- output projection (feature-major) ----
            osb = []
            for nt in range(NT_PROJ):
                psA = mm_ps.tile([128, 288], F32, tag="mm", name="mm")
                psB = mm_ps.tile([128, 288], F32, tag="mm", name="mm")
                for kc in range(KC):
                    wt = wp[kc][:, nt * 128:(nt + 1) * 128]
                    nc.tensor.matmul(psA[:, :], wt, yt[kc][:, 0:288],
                                     start=(kc == 0), stop=(kc == KC - 1))
                    nc.tensor.matmul(psB[:, :], wt, yt[kc][:, 288:576],
                                     start=(kc == 0), stop=(kc == KC - 1))
                ot = osb_p.tile([128, T], BF16, tag="osb", name="osb")
                bias = bp_sb[:, nt:nt + 1]
                nc.scalar.activation(ot[:, 0:288], psA[:, :], AF.Identity, bias=bias)
                nc.scalar.activation(ot[:, 288:576], psB[:, :], AF.Identity, bias=bias)
                osb.append(ot)

            # ---- transpose to natural layout + store ----
            for (t0, tp) in TT:
                on = onat_p.tile([128, C], BF16, tag="on", name="on")
                for cq in range(4):
                    ps = mm_ps.tile([128, 288], F32, tag="mm", name="mm")
                    nc.tensor.matmul(ps[0:tp, 0:128],
                                     osb[2 * cq][:, t0:t0 + tp],
                                     id_sb[:, 0:128], start=True, stop=True)
                    nc.tensor.matmul(ps[0:tp, 128:256],
                                     osb[2 * cq + 1][:, t0:t0 + tp],
                                     id_sb[:, 0:128], start=True, stop=True)
                    dst = on[0:tp, cq * 256:(cq + 1) * 256]
                    if cq % 2 == 0:
                        nc.scalar.activation(dst, ps[0:tp, 0:256], AF.Identity)
                    else:
                        nc.vector.tensor_copy(dst, ps[0:tp, 0:256])
                nc.sync.dma_start(
                    out_nat[mofs + t0:mofs + t0 + tp, :], on[0:tp, :]
                )

    nc.compile()
    return nc


# ---------------------------------------------------------------------------
# Host runner: cached jitted NEFF wrapper (same PJRT execution path as
# bass_utils.run_bass_kernel_spmd under axon, minus the per-call re-trace,
# the host-built zero buffers, and the replicated-weight re-uploads).
# ---------------------------------------------------------------------------

_RT = None           # runtime dict
_STATIC_DEV = None   # name -> committed device array (weights etc.)
_STATIC_KEY = None   # host copies of (w_qkv, b_qkv, w_proj, b_proj) for check


def _static_host_arrays(w_qkv, b_qkv, w_proj, b_proj):
    w16 = np.ascontiguousarray(w_qkv).astype(NPBF16)
    p16 = np.ascontiguousarray(w_proj).astype(NPBF16)
    return {
        "w_qkv": np.tile(w16, (NCORES, 1)),
        "w_proj": np.tile(p16, (NCORES, 1)),
        "b_qkv": np.tile(np.asarray(b_qkv, np.float32), NCORES),
        "b_proj": np.tile(np.asarray(b_proj, np.float32), NCORES),
        "bvr": np.tile(np.asarray(b_qkv[2 * C:3 * C], np.float32)
                       .astype(NPBF16).reshape(1, C), (NCORES, 1)),
        "ones_r": np.ones((NCORES, 128), NPBF16),
        "ones_c": np.ones((NCORES * 128, H), NPBF16),
        "ident": np.tile(np.eye(128, dtype=NPBF16), (NCORES, 1)),
    }


def _get_runtime():
    global _RT
    if _RT is not None:
        return _RT
    import jax
    import jax.numpy as jnp
    from jax.experimental.shard_map import shard_map
    from jax.sharding import Mesh, PartitionSpec, NamedSharding
    from concourse.bass2jax import (
        _bass_exec_p, install_neuronx_cc_hook, partition_id_tensor,
    )

    nc = build_program()
    install_neuronx_cc_hook()

    partition_name = nc.partition_id_tensor.name if nc.partition_id_tensor else None
    in_names, out_names, out_avals = [], [], []
    for alloc in nc.m.functions[0].allocations:
        if not isinstance(alloc, mybir.MemoryLocationSet):
            continue
        assert alloc.memorylocations
        name = alloc.memorylocations[0].name
        if alloc.kind == "ExternalInput":
            if name != partition_name:
                in_names.append(name)
        elif alloc.kind == "ExternalOutput":
            assert alloc.tensor_shape is not None and alloc.dtype is not None
            out_names.append(name)
            out_avals.append(jax.core.ShapedArray(
                tuple(alloc.tensor_shape), mybir.dt.np(alloc.dtype)))
    n_params = len(in_names)
    all_names = list(in_names) + out_names
    if partition_name is not None:
        all_names.append(partition_name)

    dbg_name = nc.dbg_addr.name if nc.dbg_addr is not None else None

    def _body(*args):
        operands = list(args)
        if partition_name is not None:
            operands.append(partition_id_tensor())
        outs = _bass_exec_p.bind(
            *operands,
            out_avals=tuple(out_avals),
            in_names=tuple(all_names),
            out_names=tuple(out_names),
            lowering_input_output_aliases=(),
            sim_require_finite=True,
            sim_require_nnan=True,
            nc=nc,
        )
        return tuple(outs)

    devices = jax.devices()[:NCORES]
    mesh = Mesh(np.asarray(devices), ("core",))
    pspec = PartitionSpec("core")
    n_out = len(out_names)
    donate = tuple(range(n_params, n_params + n_out))
    sharded = jax.jit(
        shard_map(
            _body, mesh=mesh,
            in_specs=(pspec,) * (n_params + n_out),
            out_specs=(pspec,) * n_out,
            check_rep=False,
        ),
        donate_argnums=donate,
        keep_unused=True,
    )
    out_sharding = NamedSharding(mesh, pspec)
    zero_specs = [
        (tuple([NCORES * a.shape[0]] + list(a.shape[1:])), a.dtype)
        for a in out_avals
    ]

    def _mk_zeros():
        return tuple(jnp.zeros(s, d) for s, d in zero_specs)

    zeros_fn = jax.jit(_mk_zeros, out_shardings=(out_sharding,) * n_out)

    _RT = dict(
        nc=nc, jax=jax, mesh=mesh, devices=devices,
        sharding=out_sharding, sharded=sharded, zeros_fn=zeros_fn,
        in_names=in_names, out_names=out_names, dbg_name=dbg_name,
    )
    return _RT


def _put_sharded(rt, global_np):
    """Threaded per-shard h2d; returns a committed global jax.Array."""
    from concurrent.futures import ThreadPoolExecutor
    jax = rt["jax"]
    devices = rt["devices"]
    rows = global_np.shape[0] // NCORES
    shards = [global_np[i * rows:(i + 1) * rows] for i in range(NCORES)]

    def put(i):
        return jax.device_put(shards[i], devices[i])
    with ThreadPoolExecutor(NCORES) as ex:
        bufs = list(ex.map(put, range(NCORES)))
    return jax.make_array_from_single_device_arrays(
        global_np.shape, rt["sharding"], bufs)


def _ensure_static(rt, w_qkv, b_qkv, w_proj, b_proj):
    global _STATIC_DEV, _STATIC_KEY
    key = (w_qkv, b_qkv, w_proj, b_proj)
    if _STATIC_DEV is not None and all(
        np.array_equal(a, b) for a, b in zip(_STATIC_KEY, key)
    ):
        return _STATIC_DEV
    host = _static_host_arrays(w_qkv, b_qkv, w_proj, b_proj)
    if rt["dbg_name"] is not None:
        host[rt["dbg_name"]] = np.zeros((NCORES, 2), np.uint32)
    _STATIC_DEV = {k: _put_sharded(rt, v) for k, v in host.items()}
    _STATIC_KEY = tuple(np.array(a, copy=True) for a in key)
    return _STATIC_DEV


def kernel(emb_img, w_qkv, b_qkv, w_proj, b_proj):
    from concurrent.futures import ThreadPoolExecutor
    rt = _get_runtime()

    # donated zero output buffers, created on device (async dispatch)
    zeros = rt["zeros_fn"]()

    static = _ensure_static(rt, w_qkv, b_qkv, w_proj, b_proj)

    x16 = np.ascontiguousarray(
        np.asarray(emb_img, np.float32).reshape(NCORES * M, C)).astype(NPBF16)
    x_dev = _put_sharded(rt, x16)

    args = []
    for name in rt["in_names"]:
        args.append(x_dev if name == "x_nat" else static[name])
    outs = rt["sharded"](*args, *zeros)

    out_global = outs[rt["out_names"].index("out_nat")]
    shards = sorted(out_global.addressable_shards,
                    key=lambda s: s.index[0].start or 0)
    res = np.empty((B, T, C), np.float32)
    res2 = res.reshape(NCORES, M, C)

    def fetch(i):
        res2[i] = np.asarray(shards[i].data)
    with ThreadPoolExecutor(NCORES) as ex:
        list(ex.map(fetch, range(NCORES)))
    return res


# ---- helpers for test.py (CoreSim single-core check) ----

def make_core0_map(emb_img, w_qkv, b_qkv, w_proj, b_proj):
    x16 = np.asarray(emb_img[:BPC], np.float32).reshape(M, C).astype(NPBF16)
    return {
        "x_nat": x16,
        "w_qkv": np.asarray(w_qkv, np.float32).astype(NPBF16),
        "w_proj": np.asarray(w_proj, np.float32).astype(NPBF16),
        "b_qkv": np.asarray(b_qkv, np.float32),
        "b_proj": np.asarray(b_proj, np.float32),
        "bvr": np.asarray(b_qkv[2 * C:3 * C], np.float32).astype(NPBF16).reshape(1, C),
        "ones_r": np.ones((1, 128), NPBF16),
        "ones_c": np.ones((128, H), NPBF16),
        "ident": np.eye(128, dtype=NPBF16),
    }
